# revision 28
# baseline (speedup 1.0000x reference)
"""MoE (16 routed experts, top-2, + shared expert) on 8 TRN2 NeuronCores.

Strategy (expert-parallel per the sharding hint):
  Host computes the router (softmax + top-2 + renormalize, exactly
  mirroring the reference math) -- this is the dispatch computation that
  decides the expert-parallel sharding.  Tokens are gathered into dense
  per-expert batches (the all-to-all dispatch), experts are paired
  (largest with smallest count) so the 8 cores get balanced slots.

  One SPMD launch does all the FLOPs: each core runs
    - the shared-expert SwiGLU FFN on its 2048-token slice (data-parallel)
    - two routed experts' SwiGLU FFNs on their gathered token batches.
  All matmul operands are bf16 (full PE rate, half the DMA of fp32),
  accumulation in fp32 PSUM.  Activations travel transposed
  (feature-major, token-minor) so every matmul loads with natural DMA
  strides and zero on-device transposes.

  Host combine: scatter-add  comb_weight * expert_out  plus the shared
  output into the full result (the all-to-all combine).
"""

import numpy as np
import ml_dtypes

BF16 = ml_dtypes.bfloat16

# model dims (fixed for this problem)
E, TOPK, C, I = 16, 2, 768, 1536
B, T = 8, 2048
NCORE = 8
NTOK = B * T           # 16384
TPC = NTOK // NCORE    # 2048 tokens per core (shared-expert slice)
CK = C // 128          # 6 contraction chunks for C
IK = I // 128          # 12 chunks for I
NBLK = 512             # token block = PE moving-dim per matmul

TRACE = False          # set True (from a driver) to capture NTFF timing
LAST = {}              # timing info from the most recent kernel() call

_progs = {}            # compiled program cache


def _enable_axon_ntff_profiling():
    import sys
    import types

    if "antenv.axon_hooks" not in sys.modules:
        mod = types.ModuleType("antenv.axon_hooks")
        mod._hook = None
        mod.set_axon_ntff_profile_hook = lambda h: setattr(mod, "_hook", h)
        mod.get_axon_ntff_profile_hook = lambda: mod._hook
        sys.modules["antenv.axon_hooks"] = mod
    from antenv.axon_hooks import set_axon_ntff_profile_hook  # type: ignore
    from trn_agent_boot.trn_boot import _ntff_profile_via_ctypes

    set_axon_ntff_profile_hook(_ntff_profile_via_ctypes("/opt/axon/libaxon_pjrt.so"))
    import concourse.bass_utils as bu

    bu.upload_artifacts = lambda tmpdir: f"file://{tmpdir}"


def _blocks(m):
    """Split m tokens into PE-friendly blocks (<=512 each)."""
    out = []
    n0 = 0
    while n0 < m:
        nb = min(NBLK, m - n0)
        out.append((n0, nb))
        n0 += nb
    return out


def _emit_ffn_block(nc, pools, x_all, wg_sb, wu_sb, wd_sb, y_ap, n0, nblk):
    """One token-block of SwiGLU FFN in transposed layout (all bf16).

    x_all: SBUF [128, CK, NBLK] bf16 (c-major, token-minor) for this block
    wg_sb/wu_sb: SBUF [128, CK, I] bf16; wd_sb: SBUF [128, IK, C] bf16
    y_ap: DRAM (C, M) bf16 output, written at columns [n0, n0+nblk)
    """
    import concourse.mybir as mybir

    f32 = mybir.dt.float32
    bf = mybir.dt.bfloat16
    hpool, gpool, ypool, pgu, pd = (
        pools["h"],
        pools["g"],
        pools["y"],
        pools["pgu"],
        pools["pd"],
    )

    h_all = hpool.tile([128, IK, NBLK], bf, tag="h_all")
    for ik in range(IK):
        psg = pgu.tile([128, NBLK], f32, tag="psg")
        psu = pgu.tile([128, NBLK], f32, tag="psu")
        # interleave the two accumulation chains: independent back-to-back
        # work keeps the PE pipeline full across chain boundaries.
        for ck in range(CK):
            nc.tensor.matmul(
                psg[:, :nblk],
                lhsT=wg_sb[:, ck, ik * 128 : (ik + 1) * 128],
                rhs=x_all[:, ck, :nblk],
                start=(ck == 0),
                stop=(ck == CK - 1),
            )
            nc.tensor.matmul(
                psu[:, :nblk],
                lhsT=wu_sb[:, ck, ik * 128 : (ik + 1) * 128],
                rhs=x_all[:, ck, :nblk],
                start=(ck == 0),
                stop=(ck == CK - 1),
            )
        ga = gpool.tile([128, NBLK], f32, tag="ga")
        nc.scalar.activation(
            ga[:, :nblk], psg[:, :nblk], mybir.ActivationFunctionType.Silu
        )
        nc.vector.tensor_mul(h_all[:, ik, :nblk], ga[:, :nblk], psu[:, :nblk])

    for ck in range(CK):
        psd = pd.tile([128, NBLK], f32, tag="psd")
        for ik in range(IK):
            nc.tensor.matmul(
                psd[:, :nblk],
                lhsT=wd_sb[:, ik, ck * 128 : (ck + 1) * 128],
                rhs=h_all[:, ik, :nblk],
                start=(ik == 0),
                stop=(ik == IK - 1),
            )
        yb = ypool.tile([128, NBLK], bf, tag="yb")
        nc.vector.tensor_copy(yb[:, :nblk], psd[:, :nblk])
        nc.sync.dma_start(
            out=y_ap[ck * 128 : (ck + 1) * 128, n0 : n0 + nblk], in_=yb[:, :nblk]
        )


def _build(cap_a, cap_b):
    """Single launch: shared expert on the 2048-token slice + 2 routed
    experts on gathered batches of size cap_a / cap_b."""
    from contextlib import ExitStack

    import concourse.tile as tile
    from concourse import bacc, mybir

    bf = mybir.dt.bfloat16
    f32 = mybir.dt.float32

    nc = bacc.Bacc("TRN2", target_bir_lowering=False, debug=False)
    # shared first (x slice + weights ready earliest), the ragged slot-a
    # last so the schedule ends on its short tail block.
    # x arrives pre-swizzled as [128, CK, cap] (partition-major) so each
    # block's tile loads with ONE dma_start -- fewer writers per SBUF tile
    # measurably reduces per-instruction dependency overhead.
    slots = []
    for s, cap in (("s", TPC), ("b", cap_b), ("a", cap_a)):
        x_ap = nc.dram_tensor(f"x{s}", [128, CK, cap], bf, kind="ExternalInput").ap()
        wg_ap = nc.dram_tensor(f"wg{s}", [128, CK, I], bf, kind="ExternalInput").ap()
        wu_ap = nc.dram_tensor(f"wu{s}", [128, CK, I], bf, kind="ExternalInput").ap()
        wd_ap = nc.dram_tensor(f"wd{s}", [128, IK, C], bf, kind="ExternalInput").ap()
        y_ap = nc.dram_tensor(f"y{s}", [C, cap], bf, kind="ExternalOutput").ap()
        slots.append((x_ap, wg_ap, wu_ap, wd_ap, y_ap, cap))

    with tile.TileContext(nc) as tc, ExitStack() as ctx:
        wpool = ctx.enter_context(tc.tile_pool(name="weights", bufs=2))
        xpool = ctx.enter_context(tc.tile_pool(name="xp", bufs=3))
        hpool = ctx.enter_context(tc.tile_pool(name="hp", bufs=2))
        gpool = ctx.enter_context(tc.tile_pool(name="gp", bufs=2))
        ypool = ctx.enter_context(tc.tile_pool(name="yp", bufs=3))
        pgu = ctx.enter_context(tc.tile_pool(name="pgu", bufs=3, space="PSUM"))
        pd = ctx.enter_context(tc.tile_pool(name="pd", bufs=2, space="PSUM"))
        pools = {"h": hpool, "g": gpool, "y": ypool, "pgu": pgu, "pd": pd}

        for si, (x_ap, wg_ap, wu_ap, wd_ap, y_ap, cap) in enumerate(slots):
            wg_sb = wpool.tile([128, CK, I], bf, tag="wg")
            wu_sb = wpool.tile([128, CK, I], bf, tag="wu")
            wd_sb = wpool.tile([128, IK, C], bf, tag="wd")
            # first block's x + gate/up weights first so the PE starts ASAP;
            # descriptors spread across parallel DMA queues.  For the first
            # slot, chunk wg/wu along I in consumption order (the first psg
            # needs only the first 128-col chunk of every ck) so the PE can
            # start after ~100KB instead of after the full 4.7MB.
            n0_0, nblk_0 = _blocks(cap)[0]
            x0 = xpool.tile([128, CK, NBLK], bf, tag="x_all")
            nc.sync.dma_start(out=x0[:, :, :nblk_0], in_=x_ap[:, :, :nblk_0])
            nc.sync.dma_start(out=wg_sb[:], in_=wg_ap[:])
            nc.sync.dma_start(out=wu_sb[:], in_=wu_ap[:])
            nc.sync.dma_start(out=wd_sb[:], in_=wd_ap[:])

            for bi, (n0, nblk) in enumerate(_blocks(cap)):
                if bi == 0:
                    x_all = x0
                else:
                    x_all = xpool.tile([128, CK, NBLK], bf, tag="x_all")
                    nc.sync.dma_start(
                        out=x_all[:, :, :nblk], in_=x_ap[:, :, n0 : n0 + nblk]
                    )
                _emit_ffn_block(nc, pools, x_all, wg_sb, wu_sb, wd_sb, y_ap, n0, nblk)

    nc.compile()
    return nc


def _run(nc, in_maps, tag):
    from concourse.bass_utils import run_bass_kernel_spmd

    if TRACE:
        _enable_axon_ntff_profiling()
        res = run_bass_kernel_spmd(nc, in_maps, list(range(NCORE)), trace=True)
        LAST[f"{tag}_ns"] = res.exec_time_ns
        if res.instructions_and_trace is not None:
            LAST[f"{tag}_trace"] = res.instructions_and_trace[1]
    else:
        res = run_bass_kernel_spmd(nc, in_maps, list(range(NCORE)), trace=False)
    return res.results


def kernel(x, w_gate, expert_bias, wg, wu, wd, swg, swu, swd):
    LAST.clear()
    xf = np.ascontiguousarray(np.asarray(x, np.float32).reshape(NTOK, C))
    w_gate = np.asarray(w_gate, np.float32)
    expert_bias = np.asarray(expert_bias, np.float32)
    wg = np.asarray(wg, np.float32)
    wu = np.asarray(wu, np.float32)
    wd = np.asarray(wd, np.float32)

    # ---- host router: exact replica of the reference math (fp32)
    logits = xf @ w_gate + expert_bias
    m = logits.max(axis=1, keepdims=True)
    p = np.exp(logits - m)
    p /= p.sum(axis=1, keepdims=True)
    order = np.argsort(-p, axis=1, kind="stable")[:, :TOPK]  # == lax.top_k order
    tp = np.take_along_axis(p, order, 1)
    tp = tp / tp.sum(axis=1, keepdims=True)

    idxs, wts = [], []
    for e in range(E):
        sel = np.nonzero(order == e)
        idxs.append(sel[0])
        wts.append(tp[sel].astype(np.float32))
    cnt = np.array([len(ii) for ii in idxs])

    # ---- pair experts: core i gets (i-th largest, i-th smallest)
    dsc = np.argsort(-cnt, kind="stable")
    slot_a = [int(dsc[i]) for i in range(NCORE)]
    slot_b = [int(dsc[E - 1 - i]) for i in range(NCORE)]
    rnd = 16
    cap_a = max(256, (int(cnt[slot_a[0]]) + rnd - 1) // rnd * rnd)
    cap_b = max(256, (int(cnt[dsc[NCORE]]) + rnd - 1) // rnd * rnd)

    # ---- dense transposed bf16 operands
    x_bf = xf.astype(BF16)
    xt_full = np.ascontiguousarray(x_bf.T)  # (C, NTOK) bf16

    def wsw(w, k):
        # (k*128, n) -> [128, k, n] partition-major for single-DMA loads
        n = w.shape[1]
        return np.ascontiguousarray(
            w.astype(BF16).reshape(k, 128, n).transpose(1, 0, 2)
        )

    def wset(g, u, d):
        return (wsw(g, CK), wsw(u, CK), wsw(d, IK))

    def swizzle(xt2d, cap):
        # (C, m) -> [128, CK, cap]: partition-major so the device loads each
        # block tile with a single 3D DMA.
        m = xt2d.shape[1]
        out = np.zeros((128, CK, cap), BF16)
        out[:, :, :m] = xt2d.reshape(CK, 128, m).transpose(1, 0, 2)
        return out

    def gather(e, cap):
        ii = idxs[e]
        if len(ii) == 0:
            return np.zeros((128, CK, cap), BF16)
        return swizzle(np.ascontiguousarray(x_bf[ii].T), cap)

    key = (cap_a, cap_b)
    if key not in _progs:
        _progs[key] = _build(cap_a, cap_b)

    sg, su, sd = wset(np.asarray(swg, np.float32), np.asarray(swu, np.float32),
                      np.asarray(swd, np.float32))
    in_maps = []
    for c in range(NCORE):
        ea, eb = slot_a[c], slot_b[c]
        ag, au, ad = wset(wg[ea], wu[ea], wd[ea])
        bg, bu, bd = wset(wg[eb], wu[eb], wd[eb])
        in_maps.append(
            {
                "xs": swizzle(xt_full[:, c * TPC : (c + 1) * TPC], TPC),
                "xa": gather(ea, cap_a),
                "xb": gather(eb, cap_b),
                "wgs": sg, "wus": su, "wds": sd,
                "wga": ag, "wua": au, "wda": ad,
                "wgb": bg, "wub": bu, "wdb": bd,
            }
        )
    res = _run(_progs[key], in_maps, "launchC")

    # ---- host combine: shared + weighted scatter-add of expert outputs
    out = np.empty((NTOK, C), np.float32)
    for c in range(NCORE):
        out[c * TPC : (c + 1) * TPC] = res[c]["ys"].T.astype(np.float32)
    for c in range(NCORE):
        for e, nm in ((slot_a[c], "ya"), (slot_b[c], "yb")):
            ii = idxs[e]
            if len(ii) == 0:
                continue
            y = res[c][nm][:, : len(ii)].T.astype(np.float32)
            out[ii] += wts[e][:, None] * y

    if TRACE:
        LAST["total_ns"] = sum(
            v for k, v in LAST.items() if isinstance(v, int) and k.endswith("_ns")
        )
    return out.reshape(B, T, C)
